# revision 4
# baseline (speedup 1.0000x reference)
"""Transformer block (QKV + causal MHA + proj + GELU-FF, residual) on 8 NeuronCores.

Sharding: DP over batch (2 groups of 4 cores) x TP over heads / FF-inner within
each group. Identical SPMD program on all cores; per-core differences are input
slices only. Activations are feature-major end to end; no on-device transposes.

v2 vs baseline:
- bf16 everywhere; PSUM accumulation f32.
- Weights fetched from HBM exactly once per rep (wqk/wv/w1/w2/wp resident in
  SBUF); x streamed twice (P1 at 256-token subchunks, FF1 at 512).
- Attention inner loop software-pipelined (skew 2) so exp on ACT overlaps
  score/AV matmuls on PE.
- proj and ff2 partials accumulate in one PSUM group; bf16 ReduceScatter
  chunked over 4 output-row groups overlaps P3b compute.
- Host adds x + b_ff2 (residual) during unshard.
"""
import numpy as np
import ml_dtypes

import concourse.bass as bass
import concourse.mybir as mybir
import concourse.tile as tile
from concourse import bacc
from concourse import bass_utils

B, T, C = 2, 2048, 2048
H, HD = 16, 128
F = 8192
NCORES = 8
TPG = 4                  # cores per batch group
HPC = H // TPG           # heads per core
QC = 4                   # token chunks per batch (attention q blocks)
TCH = T // QC            # 512
XCH = 256                # P1 x-streaming subchunk
NXC = T // XCH           # 8
KT = C // 128            # 16
FPC = F // TPG           # 2048 ff rows per core
FT = FPC // 128          # 16
SM_SCALE = 1.0 / float(np.sqrt(HD))
NEG = -60000.0

f32 = mybir.dt.float32
bf16 = mybir.dt.bfloat16

_CACHED_NC = None


def build_nc(rep=1):
    nc = bacc.Bacc("TRN2", target_bir_lowering=False, debug=False,
                   num_devices=NCORES)
    xb_t = nc.dram_tensor("xb", [C, T], bf16, kind="ExternalInput").ap()
    wqk_t = nc.dram_tensor("wqk", [C, 2 * HPC * HD], bf16, kind="ExternalInput").ap()
    wv_t = nc.dram_tensor("wv", [C, HPC * HD], bf16, kind="ExternalInput").ap()
    wp_t = nc.dram_tensor("wp", [HPC * HD, C], bf16, kind="ExternalInput").ap()
    w1_t = nc.dram_tensor("w1", [C, FPC], bf16, kind="ExternalInput").ap()
    b1_t = nc.dram_tensor("b1", [128, FT], f32, kind="ExternalInput").ap()
    w2_t = nc.dram_tensor("w2", [FPC, C], bf16, kind="ExternalInput").ap()
    out_t = nc.dram_tensor("outp", [TPG * 128, T], bf16, kind="ExternalOutput").ap()

    xb_v = xb_t.rearrange("(kt p) t -> p kt t", p=128)
    wqk_v = wqk_t.rearrange("(kt p) f -> p kt f", p=128)
    wv_v = wv_t.rearrange("(kt p) f -> p kt f", p=128)
    wp_v = wp_t.rearrange("(kt p) c -> p kt c", p=128)
    w1_v = w1_t.rearrange("(kt p) f -> p kt f", p=128)
    w2_v = w2_t.rearrange("(ft p) c -> p ft c", p=128)

    with tile.TileContext(nc) as tc:
        with tc.tile_pool(name="cst", bufs=1) as cst:
            b1_sb = cst.tile([128, FT], f32, name="b1_sb", tag="b1")
            nc.sync.dma_start(b1_sb[:], b1_t)

            for _rep in range(rep):
                _build_rep(nc, tc, xb_v, wqk_v, wv_v, wp_v, w1_v, w2_v, out_t,
                           b1_sb)

    nc.compile()
    return nc


def _build_rep(nc, tc, xb_v, wqk_v, wv_v, wp_v, w1_v, w2_v, out_t, b1_sb):
    with tc.tile_pool(name="pA", bufs=1) as pA, \
         tc.tile_pool(name="dram", bufs=1, space="DRAM") as dram:
        attnT = pA.tile([128, HPC, T], bf16, name="attnT", tag="attnT")
        h_sb = pA.tile([128, FT, T], bf16, name="h_sb", tag="h")

        with tc.tile_pool(name="pQKV", bufs=1) as pQKV:
            qk_sb = pQKV.tile([128, 2 * HPC, T], bf16, name="qk_sb", tag="qk")
            v_sb = pQKV.tile([128, T // 128, HPC * HD], bf16, name="v_sb",
                             tag="v")

            # ---- P1: q/k (feature-major) and v (token-major) projections
            with tc.tile_pool(name="pX", bufs=1) as pX, \
                 tc.tile_pool(name="psX", bufs=1, space="PSUM") as psX:
                wqk_sb = pX.tile([128, KT, 2 * HPC * HD], bf16, name="wqk_sb",
                                 tag="wqk")
                # per-ft slabs so the first matmul only waits ~2us, not 12
                for ft in range(2 * HPC):
                    fs = slice(ft * 128, (ft + 1) * 128)
                    nc.sync.dma_start(wqk_sb[:, :, fs], wqk_v[:, :, fs])
                wv_sb = pX.tile([128, KT, HPC * HD], bf16, name="wv_sb",
                                tag="wv")
                nc.sync.dma_start(wv_sb[:], wv_v)
                for sc in range(NXC):
                    ss = slice(sc * XCH, (sc + 1) * XCH)
                    xc = pX.tile([128, KT, XCH], bf16, name="xc", tag="xc",
                                 bufs=2)
                    nc.sync.dma_start(xc[:], xb_v[:, :, ss])
                    for ft in range(2 * HPC):
                        pt = psX.tile([128, XCH], f32, name="pmma", tag="pmma",
                                      bufs=2)
                        for k in range(KT):
                            nc.tensor.matmul(
                                pt[:], wqk_sb[:, k, ft * 128:(ft + 1) * 128],
                                xc[:, k, :], start=(k == 0), stop=(k == KT - 1))
                        nc.vector.tensor_copy(qk_sb[:, ft, ss], pt[:])
                    for ml in range(XCH // 128):
                        m = (XCH // 128) * sc + ml
                        pt = psX.tile([128, HPC * HD], f32, name="pmmb",
                                      tag="pmmb", bufs=2)
                        for k in range(KT):
                            nc.tensor.matmul(
                                pt[:], xc[:, k, ml * 128:(ml + 1) * 128],
                                wv_sb[:, k, :], start=(k == 0),
                                stop=(k == KT - 1))
                        nc.vector.tensor_copy(v_sb[:, m, :], pt[:])

            # ---- P2: causal attention for this core's heads
            # (w1 for P3a prefetches on the right side, hidden under P2)
            pF = tc.alloc_tile_pool(name="pF", bufs=1, side="right")
            w1_sb = pF.tile([128, KT, FPC], bf16, name="w1_sb", tag="w1")
            nc.sync.dma_start(w1_sb[:], w1_v)
            with tc.tile_pool(name="pP2", bufs=1) as p2w, \
                 tc.tile_pool(name="psB", bufs=1, space="PSUM") as psB:
                onesf = p2w.tile([128, 1], f32, name="onesf", tag="onesf")
                nc.gpsimd.memset(onesf[:], 1.0)
                onesr = p2w.tile([1, 128], f32, name="onesr", tag="onesr")
                nc.gpsimd.memset(onesr[:], 1.0)
                ones_col = p2w.tile([128, 1], bf16, name="ones_col", tag="oc")
                nc.vector.tensor_copy(ones_col[:], onesf[:])
                ones_row = p2w.tile([1, 128], bf16, name="ones_row", tag="or")
                nc.vector.tensor_copy(ones_row[:], onesr[:])
                masks = p2w.tile([128, QC, TCH], bf16, name="masks",
                                 tag="mask")
                nc.gpsimd.memset(masks[:], 0.0)
                for d in range(QC):
                    nc.gpsimd.affine_select(
                        out=masks[:, d, :], in_=masks[:, d, :],
                        compare_op=mybir.AluOpType.is_ge,
                        fill=NEG, base=-d * 128,
                        pattern=[[1, TCH]], channel_multiplier=-1,
                    )
                for h in range(HPC):
                    for c in range(QC):
                        cs = slice(c * TCH, (c + 1) * TCH)
                        nkt = 4 * c + 4
                        po = psB.tile([128, TCH], f32, name="po", tag="po",
                                      bufs=2)
                        psums = psB.tile([1, TCH], f32, name="psums",
                                         tag="psums", bufs=1)
                        e_tiles = {}

                        def emit_acc(ktd):
                            nc.tensor.matmul(
                                psums[:], ones_col[:], e_tiles[ktd][:],
                                start=(ktd == 0), stop=(ktd == nkt - 1))
                            nc.tensor.matmul(
                                po[:], v_sb[:, ktd, h * HD:(h + 1) * HD],
                                e_tiles[ktd][:], start=(ktd == 0),
                                stop=(ktd == nkt - 1))

                        # diag blocks (kt>=4c): columns below the causal
                        # frontier are all-masked; trim them from every op.
                        lo = {kt: max(0, (kt - 4 * c) * 128) for kt in
                              range(nkt)}

                        def emit_acc2(ktd):
                            l = lo[ktd]
                            nc.tensor.matmul(
                                psums[:, l:], ones_col[:],
                                e_tiles[ktd][:, l:],
                                start=(ktd == 0), stop=(ktd == nkt - 1))
                            nc.tensor.matmul(
                                po[:, l:], v_sb[:, ktd, h * HD:(h + 1) * HD],
                                e_tiles[ktd][:, l:], start=(ktd == 0),
                                stop=(ktd == nkt - 1))

                        for kt in range(nkt):
                            l = lo[kt]
                            pscore = psB.tile([128, TCH], f32, name="pscore",
                                              tag="pscore", bufs=4)
                            nc.tensor.matmul(
                                pscore[:, l:],
                                qk_sb[:, HPC + h, kt * 128:(kt + 1) * 128],
                                qk_sb[:, h, c * TCH + l:(c + 1) * TCH],
                                start=True, stop=True)
                            e = p2w.tile([128, TCH], bf16, name="e_sb",
                                         tag="e", bufs=5)
                            if kt >= 4 * c:
                                d = kt - 4 * c
                                nc.vector.tensor_add(pscore[:, l:],
                                                     pscore[:, l:],
                                                     masks[:, d, l:])
                            nc.scalar.activation(
                                e[:, l:], pscore[:, l:],
                                mybir.ActivationFunctionType.Exp,
                                scale=SM_SCALE)
                            e_tiles[kt] = e
                            if kt >= 3:
                                emit_acc2(kt - 3)
                        emit_acc2(nkt - 3)
                        emit_acc2(nkt - 2)
                        emit_acc2(nkt - 1)

                        recip = p2w.tile([1, TCH], bf16, name="recip",
                                         tag="recip", bufs=2)
                        with nc.allow_low_precision(
                                reason="bf16 softmax 1/sum"):
                            nc.vector.reciprocal(recip[:], psums[:])
                        pbc = psB.tile([128, TCH], f32, name="pbc", tag="pbc",
                                       bufs=1)
                        nc.tensor.matmul(pbc[:], ones_row[:], recip[:],
                                         start=True, stop=True)
                        bc = p2w.tile([128, TCH], bf16, name="bc_sb", tag="bc",
                                      bufs=2)
                        nc.vector.tensor_copy(bc[:], pbc[:])
                        nc.vector.tensor_mul(attnT[:, h, cs], po[:], bc[:])

        # ---- P3a: FF1 + exact GELU -> h  (w1 already resident via pF)
        # pG opens first so P3b's streamed w2/wp tile DMAs prefetch under P3a.
        with tc.tile_pool(name="pG", bufs=1) as pG, \
             tc.tile_pool(name="psG", bufs=1, space="PSUM") as psG:
            with tc.tile_pool(name="pFx", bufs=1) as pFx, \
                 tc.tile_pool(name="psF", bufs=1, space="PSUM") as psF:
                for c in range(QC):
                    cs = slice(c * TCH, (c + 1) * TCH)
                    xc = pFx.tile([128, KT, TCH], bf16, name="xc2", tag="xc2",
                                  bufs=2)
                    nc.sync.dma_start(xc[:], xb_v[:, :, cs])
                    for f in range(FT):
                        ph = psF.tile([128, TCH], f32, name="pmm3",
                                      tag="pmm3", bufs=3)
                        for k in range(KT):
                            nc.tensor.matmul(
                                ph[:], w1_sb[:, k, f * 128:(f + 1) * 128],
                                xc[:, k, :], start=(k == 0),
                                stop=(k == KT - 1))
                        nc.scalar.activation(
                            h_sb[:, f, cs], ph[:],
                            mybir.ActivationFunctionType.Gelu,
                            bias=b1_sb[:, f:f + 1], scale=1.0)
            pF.release()

            # ---- P3b: proj + FF2 fused accumulation, chunked bf16 RS
            for g in range(TPG):
                rs_in = dram.tile([TPG * 128, T], bf16, name="rs_in",
                                  tag="rsi", bufs=2)
                for col in range(TPG):
                    co = TPG * g + col
                    w2t = pG.tile([128, FT, 128], bf16, name="w2t", tag="w2t",
                                  bufs=3)
                    nc.sync.dma_start(w2t[:],
                                      w2_v[:, :, co * 128:(co + 1) * 128])
                    wpt = pG.tile([128, TPG, 128], bf16, name="wpt",
                                  tag="wpt", bufs=3)
                    nc.sync.dma_start(wpt[:],
                                      wp_v[:, :, co * 128:(co + 1) * 128])
                    o_sb = pG.tile([128, T], bf16, name="o_sb", tag="o",
                                   bufs=2)
                    for c in range(QC):
                        cs = slice(c * TCH, (c + 1) * TCH)
                        pout = psG.tile([128, TCH], f32, name="pout",
                                        tag="pout", bufs=2)
                        for k4 in range(TPG):
                            nc.tensor.matmul(
                                pout[:], wpt[:, k4, :],
                                attnT[:, k4, cs], start=(k4 == 0), stop=False)
                        for f in range(FT):
                            nc.tensor.matmul(
                                pout[:], w2t[:, f, :],
                                h_sb[:, f, cs], start=False,
                                stop=(f == FT - 1))
                        nc.vector.tensor_copy(o_sb[:, cs], pout[:])
                    nc.sync.dma_start(rs_in[col * 128:(col + 1) * 128, :],
                                      o_sb[:])
                rs_out = dram.tile([128, T], bf16, name="rs_out", tag="rso",
                                   bufs=2)
                nc.gpsimd.collective_compute(
                    "ReduceScatter", mybir.AluOpType.add,
                    replica_groups=[[0, 1, 2, 3], [4, 5, 6, 7]],
                    ins=[rs_in.opt()], outs=[rs_out.opt()])
                nc.sync.dma_start(out_t[g * 128:(g + 1) * 128, :], rs_out[:])


def make_in_maps(x, w_qkv, w_proj, w_ff1, b_ff1, w_ff2):
    in_maps = []
    asc = np.ascontiguousarray
    bf = ml_dtypes.bfloat16
    for r in range(NCORES):
        b, hg = r // TPG, r % TPG
        q_cols = w_qkv[:, hg * 512:(hg + 1) * 512]
        k_cols = w_qkv[:, C + hg * 512:C + (hg + 1) * 512]
        v_cols = w_qkv[:, 2 * C + hg * 512:2 * C + (hg + 1) * 512]
        in_maps.append({
            "xb": asc(x[b].T).astype(bf),
            "wqk": asc(np.concatenate([q_cols, k_cols], axis=1)).astype(bf),
            "wv": asc(v_cols).astype(bf),
            "wp": asc(w_proj[hg * 512:(hg + 1) * 512, :]).astype(bf),
            "w1": asc(w_ff1[:, hg * FPC:(hg + 1) * FPC]).astype(bf),
            "b1": asc(b_ff1[hg * FPC:(hg + 1) * FPC].reshape(FT, 128).T),
            "w2": asc(w_ff2[hg * FPC:(hg + 1) * FPC, :]).astype(bf),
        })
    return in_maps


def assemble(results, x, b_ff2):
    out = np.empty((B, T, C), np.float32)
    for r in range(NCORES):
        b, i = r // TPG, r % TPG
        o = np.asarray(results[r]["outp"]).astype(np.float32)  # [512, T]
        for g in range(TPG):
            out[b, :, g * 512 + i * 128:g * 512 + (i + 1) * 128] = \
                o[g * 128:(g + 1) * 128, :].T
    out += x + b_ff2
    return out


def kernel(x, w_qkv, w_proj, w_ff1, b_ff1, w_ff2, b_ff2):
    global _CACHED_NC
    x = np.asarray(x, np.float32)
    if _CACHED_NC is None:
        _CACHED_NC = build_nc()
    in_maps = make_in_maps(x, np.asarray(w_qkv, np.float32),
                           np.asarray(w_proj, np.float32),
                           np.asarray(w_ff1, np.float32),
                           np.asarray(b_ff1, np.float32),
                           np.asarray(w_ff2, np.float32))
    res = bass_utils.run_bass_kernel_spmd(_CACHED_NC, in_maps,
                                          core_ids=list(range(NCORES)))
    return assemble(res.results, x, np.asarray(b_ff2, np.float32))
